# revision 1
# baseline (speedup 1.0000x reference)
"""Trainium2 Bass kernel: GroupNorm + single-head self-attention block.

Reference computation (per batch element b):
    xn  = GroupNorm(x)                      # [C, N]  C=256, N=4096, 8 groups
    q,k,v = w_qkv @ xn (split)              # each [C, N]
    s   = (q^T k) * C^-0.5                  # [N, N]
    p   = softmax(s, axis=-1)
    out = v @ p^T                           # [C, N]
    y   = x + w_proj @ out + b_proj

Sharding: data-parallel over batch B=4 across 8 cores, 2 cores per batch
element.  Each core handles NQ=2048 of the 4096 queries and redundantly
computes GroupNorm/K/V for its batch element (cheap), so there are no
collectives at all.  SPMD trick: the host rolls x along N per core so the
core's query half is always columns [0, NQ) — key order is consistently
permuted in both k and v, which leaves the attention output unchanged.

Layout choices (per core):
  - scores are computed TRANSPOSED: s_T[m, n] (keys on partitions) via
    matmul(lhsT=k[c, m_blk], rhs=q[c, n_tile]).  exp() on ScalarE then doubles
    as the mandatory PSUM->SBUF evacuation.
  - softmax denominator: VectorE folds the 32 exp tiles pairwise, then four
    tiny column-sum matmuls + VectorE reciprocal + identity-matmul transpose
    produce 1/den without any Ln/Exp activation-table switches.
  - v is produced directly transposed (v_T[m, c]) by using xn as the matmul
    stationary operand, so the attention output matmul needs no transposes.
  - the 1/den scale is applied after the output projection (frees the
    attention-output PSUM banks immediately, off the TensorE critical path).
  - all big matmuls run in float32r (fp32 storage, ~4x fp32 throughput).
"""

import functools

import numpy as np

C = 256
N = 4096
NQ = 2048  # queries per core
G = 8  # groupnorm groups
CB = 2  # channel blocks of 128
NT = NQ // 512  # query tiles per core
MB = N // 128  # key blocks
EPS = 1e-5
USE_F32R = True
DEN_VECTOR = True  # softmax denominator: VectorE fold instead of TensorE ones-matmul

_GRAPH = None


def _build_graph(repeats=1):
    import concourse.bass as bass
    import concourse.mybir as mybir
    from concourse import bacc, tile

    dt = mybir.dt
    f32 = dt.float32
    fr = dt.float32r if USE_F32R else dt.float32
    AF = mybir.ActivationFunctionType
    Alu = mybir.AluOpType

    nc = bacc.Bacc("TRN2", target_bir_lowering=False, debug=False, num_devices=8)

    x_d = nc.declare_dram_parameter("x", [C, N], f32, isOutput=False)
    wq_d = nc.declare_dram_parameter("w_qT", [C, C], fr, isOutput=False)
    wk_d = nc.declare_dram_parameter("w_kT", [C, C], fr, isOutput=False)
    wv_d = nc.declare_dram_parameter("w_vT", [C, C], fr, isOutput=False)
    wp_d = nc.declare_dram_parameter("w_pT", [C, C], fr, isOutput=False)
    gam_d = nc.declare_dram_parameter("gamma", [C, 1], f32, isOutput=False)
    bet_d = nc.declare_dram_parameter("beta", [C, 1], f32, isOutput=False)
    bp_d = nc.declare_dram_parameter("b_proj", [C, 1], f32, isOutput=False)
    g_d = nc.declare_dram_parameter("G", [C, G], f32, isOutput=False)
    gt_d = nc.declare_dram_parameter("GT", [G, C], f32, isOutput=False)
    onc_d = nc.declare_dram_parameter("ones_col", [128, 2], fr, isOutput=False)
    onr_d = nc.declare_dram_parameter("ones_row", [1, 128], fr, isOutput=False)
    id_d = nc.declare_dram_parameter("ident", [128, 128], fr, isOutput=False)
    out_d = nc.declare_dram_parameter("out", [C, NQ], f32, isOutput=True)


    with tile.TileContext(nc) as tc:
        with tc.tile_pool(name="pers", bufs=1) as pers:
            # ---- persistent SBUF tiles ----
            x_sb = [pers.tile([128, N], f32, name=f"x{cb}", tag=f"x{cb}") for cb in range(CB)]
            xn_sb = [pers.tile([128, N], fr, name=f"xn{cb}", tag=f"xn{cb}") for cb in range(CB)]
            k_sb = [pers.tile([128, N], fr, name=f"k{cb}", tag=f"k{cb}") for cb in range(CB)]
            q_sb = [pers.tile([128, NQ], fr, name=f"q{cb}", tag=f"q{cb}") for cb in range(CB)]
            vT_sb = pers.tile([128, MB * C], fr, name="vT", tag="vT")
            wq_sb = [pers.tile([128, C], fr, name=f"wq{cb}", tag=f"wq{cb}") for cb in range(CB)]
            wk_sb = [pers.tile([128, C], fr, name=f"wk{cb}", tag=f"wk{cb}") for cb in range(CB)]
            wv_sb = [pers.tile([128, C], fr, name=f"wv{cb}", tag=f"wv{cb}") for cb in range(CB)]
            wp_sb = [pers.tile([128, C], fr, name=f"wp{cb}", tag=f"wp{cb}") for cb in range(CB)]
            gam_sb = [pers.tile([128, 1], f32, name=f"gam{cb}", tag=f"gam{cb}") for cb in range(CB)]
            bet_sb = [pers.tile([128, 1], f32, name=f"bet{cb}", tag=f"bet{cb}") for cb in range(CB)]
            bp_sb = [pers.tile([128, 1], f32, name=f"bp{cb}", tag=f"bp{cb}") for cb in range(CB)]
            g_sb = [pers.tile([128, G], f32, name=f"g{cb}", tag=f"g{cb}") for cb in range(CB)]
            gt_sb = [pers.tile([G, 128], f32, name=f"gt{cb}", tag=f"gt{cb}") for cb in range(CB)]
            ones_col = pers.tile([128, 2], fr, name="ones_col", tag="ones_col")
            ones_row = pers.tile([1, 128], fr, name="ones_row", tag="ones_row")
            ident_sb = pers.tile([128, 128], fr, name="ident", tag="ident")
            psum_part = [pers.tile([128, 4], f32, name=f"psm{cb}", tag=f"psm{cb}") for cb in range(CB)]
            psq_part = [pers.tile([128, 4], f32, name=f"psq{cb}", tag=f"psq{cb}") for cb in range(CB)]
            stats_sb = [pers.tile([128, 2], f32, name=f"st{cb}", tag=f"st{cb}") for cb in range(CB)]
            mexp_sb = pers.tile([G, 2], f32, name="mexp", tag="mexp")
            musq_sb = pers.tile([G, 1], f32, name="musq", tag="musq")
            var_sb = pers.tile([G, 1], f32, name="var", tag="var")
            lnv_sb = pers.tile([G, 1], f32, name="lnv", tag="lnv")
            negmu_sb = pers.tile([G, 1], f32, name="negmu", tag="negmu")
            eps_sb = pers.tile([G, 1], f32, name="eps", tag="eps")
            rs2_sb = pers.tile([G, 2], f32, name="rs2", tag="rs2")
            ab_sb = [pers.tile([128, 2], f32, name=f"ab{cb}", tag=f"ab{cb}") for cb in range(CB)]
            a_sb = [pers.tile([128, 1], f32, name=f"a{cb}", tag=f"a{cb}") for cb in range(CB)]
            bvec_sb = [pers.tile([128, 1], f32, name=f"b{cb}", tag=f"b{cb}") for cb in range(CB)]
            tmp_sb = [pers.tile([128, 1], f32, name=f"tmp{cb}", tag=f"tmp{cb}") for cb in range(CB)]
            nc.sync.dma_start(ones_col[:], onc_d[:, :])
            nc.sync.dma_start(ones_row[:], onr_d[:, :])
            nc.sync.dma_start(ident_sb[:], id_d[:, :])
            nc.gpsimd.memset(eps_sb[:], EPS)

            for _rep in range(repeats):

                # ---- input DMA (x chunked so stats can start early) ----
                NCH = 4
                CHW = N // NCH
                for ch in range(NCH):
                    for cb in range(CB):
                        eng = nc.sync if cb == 0 else nc.gpsimd
                        eng.dma_start(
                            x_sb[cb][:, ch * CHW : (ch + 1) * CHW],
                            x_d[cb * 128 : (cb + 1) * 128, ch * CHW : (ch + 1) * CHW],
                        )
                for cb in range(CB):
                    sl = slice(cb * 128, (cb + 1) * 128)
                    nc.sync.dma_start(wq_sb[cb][:], wq_d[sl, :])
                    nc.sync.dma_start(wk_sb[cb][:], wk_d[sl, :])
                    nc.sync.dma_start(wv_sb[cb][:], wv_d[sl, :])
                    nc.sync.dma_start(wp_sb[cb][:], wp_d[sl, :])
                    nc.sync.dma_start(gam_sb[cb][:], gam_d[sl, :])
                    nc.sync.dma_start(bet_sb[cb][:], bet_d[sl, :])
                    nc.sync.dma_start(bp_sb[cb][:], bp_d[sl, :])
                    nc.sync.dma_start(g_sb[cb][:], g_d[sl, :])
                    nc.sync.dma_start(gt_sb[cb][:], gt_d[:, sl])

                # ---- GroupNorm statistics ----
                # per-partition sum (VectorE) and sum-of-squares (ScalarE, the
                # Square output doubles as scratch that xn later overwrites)
                for cb in range(CB):
                    for ch in range(NCH):
                        xa = x_sb[cb][:, ch * CHW : (ch + 1) * CHW]
                        nc.vector.reduce_sum(
                            psum_part[cb][:, ch : ch + 1], xa, axis=mybir.AxisListType.X
                        )
                        nc.scalar.activation(
                            xn_sb[cb][:, ch * CHW : (ch + 1) * CHW],
                            xa,
                            AF.Square,
                            accum_out=psq_part[cb][:, ch : ch + 1],
                        )
                    nc.vector.reduce_sum(
                        stats_sb[cb][:, 0:1], psum_part[cb][:], axis=mybir.AxisListType.X
                    )
                    nc.vector.reduce_sum(
                        stats_sb[cb][:, 1:2], psq_part[cb][:], axis=mybir.AxisListType.X
                    )

                with tc.tile_pool(name="ps_gn", bufs=1, space="PSUM") as ps_gn:
                    ps_g = ps_gn.tile([G, 2], f32, name="ps_g", tag="ps_g")
                    for cb in range(CB):
                        nc.tensor.matmul(
                            ps_g[:],
                            g_sb[cb][:],
                            stats_sb[cb][:],
                            start=(cb == 0),
                            stop=(cb == CB - 1),
                        )
                    # one DVE burst: copy stats, negvar = mu^2 - E[x^2], -mu
                    nc.vector.tensor_copy(mexp_sb[:], ps_g[:])
                    nc.vector.scalar_tensor_tensor(
                        var_sb[:], mexp_sb[:, 0:1], mexp_sb[:, 0:1],
                        mexp_sb[:, 1:2], op0=Alu.mult, op1=Alu.subtract,
                    )
                    nc.vector.tensor_scalar_mul(negmu_sb[:], mexp_sb[:, 0:1], -1.0)
                    # one ACT burst: rstd = exp(-0.5*ln(var+eps)); rs2b = -mu*rstd
                    nc.scalar.activation(
                        lnv_sb[:], var_sb[:], AF.Ln, bias=eps_sb[:], scale=-1.0
                    )
                    nc.scalar.activation(rs2_sb[:, 0:1], lnv_sb[:], AF.Exp, scale=-0.5)
                    nc.scalar.activation(
                        rs2_sb[:, 1:2], rs2_sb[:, 0:1], AF.Identity, scale=negmu_sb[:]
                    )
                    for cb in range(CB):
                        ps_ab = ps_gn.tile([128, 2], f32, name="ps_ab", tag="ps_ab")
                        nc.tensor.matmul(
                            ps_ab[:], gt_sb[cb][:], rs2_sb[:], start=True, stop=True
                        )
                        nc.vector.tensor_mul(a_sb[cb][:], ps_ab[:, 0:1], gam_sb[cb][:])
                        nc.vector.scalar_tensor_tensor(
                            bvec_sb[cb][:], ps_ab[:, 1:2], gam_sb[cb][:],
                            bet_sb[cb][:], op0=Alu.mult, op1=Alu.add,
                        )
                        # xn = a*x + b  (per-partition scale/bias), chunked
                        # small-first so downstream qkv matmuls start sooner
                        for c0, c1 in ((0, 512), (512, 1024), (1024, 2048), (2048, 4096)):
                            nc.scalar.activation(
                                xn_sb[cb][:, c0:c1],
                                x_sb[cb][:, c0:c1],
                                AF.Identity,
                                bias=bvec_sb[cb][:],
                                scale=a_sb[cb][:],
                            )

                # ---- QKV projections ----
                with tc.tile_pool(name="ps_qkv", bufs=3, space="PSUM") as ps_qkv:
                    # k[o, m] — natural layout, o on partitions
                    for ob in range(CB):
                        for mt in range(N // 512):
                            ps = ps_qkv.tile([128, 512], f32, name="ps_k", tag="ps_k")
                            for cb in range(CB):
                                nc.tensor.matmul(
                                    ps[:],
                                    wk_sb[cb][:, ob * 128 : (ob + 1) * 128],
                                    xn_sb[cb][:, mt * 512 : (mt + 1) * 512],
                                    start=(cb == 0),
                                    stop=(cb == CB - 1),
                                )
                            nc.vector.tensor_copy(k_sb[ob][:, mt * 512 : (mt + 1) * 512], ps[:])
                    # q[o, n] for this core's query half
                    for ob in range(CB):
                        for ntt in range(NQ // 512):
                            ps = ps_qkv.tile([128, 512], f32, name="ps_k", tag="ps_k")
                            for cb in range(CB):
                                nc.tensor.matmul(
                                    ps[:],
                                    wq_sb[cb][:, ob * 128 : (ob + 1) * 128],
                                    xn_sb[cb][:, ntt * 512 : (ntt + 1) * 512],
                                    start=(cb == 0),
                                    stop=(cb == CB - 1),
                                )
                            nc.vector.tensor_copy(q_sb[ob][:, ntt * 512 : (ntt + 1) * 512], ps[:])
                    # v_T[m, o] — produced directly transposed (xn is stationary)
                    for mb in range(MB):
                        ps = ps_qkv.tile([128, C], f32, name="ps_v", tag="ps_v")
                        for cb in range(CB):
                            nc.tensor.matmul(
                                ps[:],
                                xn_sb[cb][:, mb * 128 : (mb + 1) * 128],
                                wv_sb[cb][:],
                                start=(cb == 0),
                                stop=(cb == CB - 1),
                            )
                        nc.scalar.copy(vT_sb[:, mb * C : (mb + 1) * C], ps[:])

                # ---- attention + projection, tiled over 512-query tiles ----
                with tc.tile_pool(name="pT", bufs=6) as pT_pool, \
                     tc.tile_pool(name="att_sb", bufs=2) as att_sb, \
                     tc.tile_pool(name="y_sb", bufs=2) as y_pool, \
                     tc.tile_pool(name="ps_s", bufs=3, space="PSUM") as ps_s_pool, \
                     tc.tile_pool(name="ps_o", bufs=1, space="PSUM") as ps_o_pool, \
                     tc.tile_pool(name="ps_bc", bufs=1, space="PSUM") as ps_bc_pool, \
                     tc.tile_pool(name="ps_pj", bufs=2, space="PSUM") as ps_pj_pool:
                    for nt in range(NT):
                        qs = slice(nt * 512, (nt + 1) * 512)
                        if DEN_VECTOR:
                            acc = [
                                att_sb.tile([128, 512], fr, name=f"acc{i}", tag=f"acc{i}")
                                for i in range(2)
                            ]
                        ps_out = [
                            ps_o_pool.tile([128, 512], f32, name=f"ps_out{cb}", tag=f"ps_out{cb}")
                            for cb in range(CB)
                        ]
                        for mb in range(MB):
                            ps_s = ps_s_pool.tile([128, 512], f32, name="ps_s", tag="ps_s")
                            for cb in range(CB):
                                nc.tensor.matmul(
                                    ps_s[:],
                                    k_sb[cb][:, mb * 128 : (mb + 1) * 128],
                                    q_sb[cb][:, qs],
                                    start=(cb == 0),
                                    stop=(cb == CB - 1),
                                )
                            pT = pT_pool.tile([128, 512], fr, name="pT", tag="pT")
                            nc.scalar.activation(pT[:], ps_s[:], AF.Exp)
                            for cb in range(CB):
                                nc.tensor.matmul(
                                    ps_out[cb][:],
                                    vT_sb[:, mb * C + cb * 128 : mb * C + (cb + 1) * 128],
                                    pT[:],
                                    start=(mb == 0),
                                    stop=(mb == MB - 1),
                                )
                            if DEN_VECTOR:
                                if mb == 0:
                                    nc.vector.tensor_copy(acc[0][:], pT[:])
                                else:
                                    nc.vector.tensor_add(
                                        acc[mb % 2][:], acc[(mb - 1) % 2][:], pT[:]
                                    )
                            else:
                                nc.tensor.matmul(
                                    ps_den[:],
                                    ones_col[:],
                                    pT[:],
                                    start=(mb == 0),
                                    stop=(mb == MB - 1),
                                )
                        # den[n] as [128, 4] (4 query quarters on partitions):
                        # col-sum matmuls over the folded exp accumulator, then
                        # 1/den on VectorE (avoids Ln/Exp table switches), then
                        # transpose back to a row via identity matmuls.
                        af = acc[(MB - 1) % 2]
                        ps_den = ps_bc_pool.tile([128, 8], f32, name="ps_den", tag="ps_bc")
                        for j in range(4):
                            nc.tensor.matmul(
                                ps_den[:, 2 * j : 2 * j + 2],
                                af[:, j * 128 : (j + 1) * 128],
                                ones_col[:],
                                start=True, stop=True,
                            )
                        r4 = att_sb.tile([128, 8], fr, name="r4", tag="r4")
                        with nc.allow_low_precision(reason="softmax recip in f32r"):
                            nc.vector.reciprocal(r4[:], ps_den[:])
                        ps_tr = ps_bc_pool.tile([1, 512], f32, name="ps_tr", tag="ps_bc")
                        for j in range(4):
                            nc.tensor.matmul(
                                ps_tr[:, j * 128 : (j + 1) * 128],
                                r4[:, 2 * j : 2 * j + 1],
                                ident_sb[:],
                                start=True, stop=True,
                            )
                        r_row = att_sb.tile([1, 512], fr, name="r_row", tag="r_row")
                        nc.vector.tensor_copy(r_row[:], ps_tr[:])
                        ps_bc = ps_bc_pool.tile([128, 512], f32, name="ps_bc", tag="ps_bc")
                        nc.tensor.matmul(
                            ps_bc[:], ones_row[:], r_row[:], start=True, stop=True
                        )
                        r_sb = att_sb.tile([128, 512], f32, name="r_sb", tag="r_sb")
                        nc.scalar.activation(r_sb[:], ps_bc[:], AF.Identity)
                        # plain evacuation (frees ps_out banks immediately;
                        # the 1/den scale moves to after the projection)
                        out_s = [
                            att_sb.tile([128, 512], fr, name=f"out_s{cb}", tag=f"out_s{cb}")
                            for cb in range(CB)
                        ]
                        for cb in range(CB):
                            nc.vector.tensor_copy(out_s[cb][:], ps_out[cb][:])
                        # final projection + bias + residual
                        for ob in range(CB):
                            ps_pj = ps_pj_pool.tile([128, 512], f32, name="ps_pj", tag="ps_pj")
                            for cb in range(CB):
                                nc.tensor.matmul(
                                    ps_pj[:],
                                    wp_sb[cb][:, ob * 128 : (ob + 1) * 128],
                                    out_s[cb][:],
                                    start=(cb == 0),
                                    stop=(cb == CB - 1),
                                )
                            t1 = y_pool.tile([128, 512], f32, name="t1", tag="t1")
                            nc.vector.tensor_mul(t1[:], ps_pj[:], r_sb[:])
                            y1 = y_pool.tile([128, 512], f32, name="y1", tag="y1")
                            nc.scalar.activation(
                                y1[:], t1[:], AF.Identity, bias=bp_sb[ob][:]
                            )
                            y2 = y_pool.tile([128, 512], f32, name="y2", tag="y2")
                            nc.vector.tensor_add(y2[:], y1[:], x_sb[ob][:, qs])
                            nc.sync.dma_start(out_d[ob * 128 : (ob + 1) * 128, qs], y2[:])
    nc.compile()
    nc.finalize()
    return nc


def _get_graph():
    global _GRAPH
    if _GRAPH is None:
        _GRAPH = _build_graph()
    return _GRAPH


def _host_inputs(x, gamma, beta, w_qkv, w_proj, b_proj):
    f = np.float32
    scale = C ** -0.5
    wq = np.ascontiguousarray((w_qkv[0:C] * scale).T, dtype=f)
    wk = np.ascontiguousarray(w_qkv[C : 2 * C].T, dtype=f)
    wv = np.ascontiguousarray(w_qkv[2 * C : 3 * C].T, dtype=f)
    wp = np.ascontiguousarray(w_proj.T, dtype=f)
    gm = np.zeros((C, G), dtype=f)
    gm[np.arange(C), np.arange(C) // (C // G)] = 1.0 / float((C // G) * N)
    gmT = np.ascontiguousarray((gm != 0).astype(f).T)
    com = {
        "w_qT": wq,
        "w_kT": wk,
        "w_vT": wv,
        "w_pT": wp,
        "gamma": np.ascontiguousarray(gamma.reshape(C, 1), dtype=f),
        "beta": np.ascontiguousarray(beta.reshape(C, 1), dtype=f),
        "b_proj": np.ascontiguousarray(b_proj.reshape(C, 1), dtype=f),
        "G": gm,
        "GT": gmT,
        "ones_col": np.ones((128, 2), dtype=f),
        "ones_row": np.ones((1, 128), dtype=f),
        "ident": np.eye(128, dtype=f),
    }
    in_maps = []
    B = x.shape[0]
    for j in range(8):
        b, h = j // 2, j % 2
        x2 = np.asarray(x[b], dtype=f).reshape(C, N)
        xr = np.ascontiguousarray(np.roll(x2, -h * NQ, axis=1))
        in_maps.append({"x": xr, **com})
    return in_maps


def kernel(x, gamma, beta, w_qkv, w_proj, b_proj):
    from concourse.bass_utils import run_bass_kernel_spmd

    x = np.asarray(x)
    B, _, H, W = x.shape
    nc = _get_graph()
    in_maps = _host_inputs(x, gamma, beta, w_qkv, w_proj, b_proj)
    res = run_bass_kernel_spmd(nc, in_maps, core_ids=list(range(8)))
    y = np.empty((B, C, N), dtype=np.float32)
    for j in range(8):
        b, h = j // 2, j % 2
        y[b][:, h * NQ : (h + 1) * NQ] = res.results[j]["out"]
    return y.reshape(B, C, H, W)



# revision 7
# speedup vs baseline: 1.2493x; 1.2493x over previous
"""Trainium2 Bass kernel: GroupNorm + single-head self-attention block.

Reference computation (per batch element b):
    xn  = GroupNorm(x)                      # [C, N]  C=256, N=4096, 8 groups
    q,k,v = w_qkv @ xn (split)              # each [C, N]
    s   = (q^T k) * C^-0.5                  # [N, N]
    p   = softmax(s, axis=-1)
    out = v @ p^T                           # [C, N]
    y   = x + w_proj @ out + b_proj

Sharding: data-parallel over batch B=4 across 8 cores, 2 cores per batch
element.  Each core handles NQ=2048 of the 4096 queries and redundantly
computes GroupNorm/K/V for its batch element.  SPMD trick: the host rolls
x along N per core so the core's query half is always columns [0, NQ).

fp8 design (v2):
  - q/k/v/xn and all conv1x1 weights are quantized to fp8e4m3 on chip;
    every big matmul runs in DoubleRow perf mode (K=256 contracted in a
    single pass, 0.5 cycles/row) with 3D [128, 2, F] access patterns.
  - scores are computed transposed s_T[m, n] (keys on partitions) via
    matmul(lhsT=k8 block, rhs=q8 tile); PSUM holds the raw q.k dot
    product and the mandatory C^-0.5 scale plus a constant exp-shift
    ride the Exp activation for free:  pT = exp(s/16 - EXPC), emitted
    directly in fp8 (values bounded ~e^4 << 240-sat).  The constant
    shift cancels in the softmax ratio.
  - softmax denominator comes from a ones-lhsT DoubleRow matmul on
    TensorE accumulated across all key pairs (no DVE folds at all);
    1/den via the fast custom-DVE reciprocal, broadcast to 128
    partitions with a tiny ones-column matmul.
  - the 1/den scale is applied during the attention-output PSUM
    evacuation (DVE tensor_mul reading two PSUM tiles), which also
    quantizes to fp8 for the DoubleRow output projection.
  - GroupNorm: per-partition sums on DVE, sum-of-squares via ACT Square
    accum (prologue-idle engine), xn=a*x+b emitted as fp8 split between
    DVE and GPSIMD.
"""

import numpy as np

C = 256
N = 4096
NQ = 2048  # queries per core
G = 8  # groupnorm groups
CB = 2  # channel blocks of 128
NT = NQ // 512  # query tiles per core
MB = N // 128  # key blocks
PAIRS = MB // 2  # key pair-blocks (256 keys each)
EPS = 1e-5
SCL = C ** -0.5  # folded into the Exp activation scale
EXPC = 2.0  # constant exp shift (cancels in softmax); keeps pT in fp8 range

_GRAPH = None


def _build_graph(repeats=1):
    import concourse.bass as bass
    import concourse.mybir as mybir
    from concourse import bacc, tile

    dt = mybir.dt
    f32 = dt.float32
    fr = dt.float32r
    f8 = dt.float8e4
    AF = mybir.ActivationFunctionType
    Alu = mybir.AluOpType
    DR = mybir.MatmulPerfMode.DoubleRow

    nc = bacc.Bacc("TRN2", target_bir_lowering=False, debug=False, num_devices=8)

    x_d = nc.declare_dram_parameter("x", [C, N], f32, isOutput=False)
    wq_d = nc.declare_dram_parameter("wq8", [128, CB, C], f8, isOutput=False)
    wk_d = nc.declare_dram_parameter("wk8", [128, CB, C], f8, isOutput=False)
    wv_d = nc.declare_dram_parameter("wv8", [128, CB, C], f8, isOutput=False)
    wp_d = nc.declare_dram_parameter("wp8", [128, CB, C], f8, isOutput=False)
    gam_d = nc.declare_dram_parameter("gamma", [C, 1], f32, isOutput=False)
    bet_d = nc.declare_dram_parameter("beta", [C, 1], f32, isOutput=False)
    bp_d = nc.declare_dram_parameter("b_proj", [C, 1], f32, isOutput=False)
    g_d = nc.declare_dram_parameter("G", [C, G], f32, isOutput=False)
    on8_d = nc.declare_dram_parameter("ones8", [128, CB * 16], f8, isOutput=False)
    onr_d = nc.declare_dram_parameter("ones_row", [1, 128], fr, isOutput=False)
    gt_d = nc.declare_dram_parameter("GT", [G, C], f32, isOutput=False)
    out_d = nc.declare_dram_parameter("out", [C, NQ], f32, isOutput=True)

    with tile.TileContext(nc) as tc:
        with tc.tile_pool(name="pers", bufs=1) as pers:
            # ---- persistent SBUF tiles ----
            x_sb = [pers.tile([128, N], f32, name=f"x{cb}", tag=f"x{cb}") for cb in range(CB)]
            xn8 = pers.tile([128, CB, N], f8, name="xn8", tag="xn8")
            k8 = pers.tile([128, CB, N], f8, name="k8", tag="k8")
            q8 = pers.tile([128, CB, NQ], f8, name="q8", tag="q8")
            vT8 = pers.tile([128, MB, C], f8, name="vT8", tag="vT8")
            sq8 = pers.tile([128, N], f8, name="sq8", tag="sq8")  # Square dump
            wq_sb = pers.tile([128, CB, C], f8, name="wq8s", tag="wq8s")
            wk_sb = pers.tile([128, CB, C], f8, name="wk8s", tag="wk8s")
            wv_sb = pers.tile([128, CB, C], f8, name="wv8s", tag="wv8s")
            wp_sb = pers.tile([128, CB, C], f8, name="wp8s", tag="wp8s")
            gam_sb = [pers.tile([128, 1], f32, name=f"gam{cb}", tag=f"gam{cb}") for cb in range(CB)]
            bet_sb = [pers.tile([128, 1], f32, name=f"bet{cb}", tag=f"bet{cb}") for cb in range(CB)]
            bp_sb = [pers.tile([128, 1], f32, name=f"bp{cb}", tag=f"bp{cb}") for cb in range(CB)]
            g_sb = [pers.tile([128, G], f32, name=f"g{cb}", tag=f"g{cb}") for cb in range(CB)]
            gt_sb = [pers.tile([G, 128], f32, name=f"gt{cb}", tag=f"gt{cb}") for cb in range(CB)]
            # ones for the denominator matmul: slice [:, :, 0:1] has a
            # 16-byte dim1 stride (DoubleRow alignment requirement)
            ones8 = pers.tile([128, CB, 16], f8, name="ones8", tag="ones8")
            ones_row = pers.tile([1, 128], fr, name="ones_row", tag="ones_row")
            psum_part = [pers.tile([128, 4], f32, name=f"psm{cb}", tag=f"psm{cb}") for cb in range(CB)]
            psq_part = [pers.tile([128, 4], f32, name=f"psq{cb}", tag=f"psq{cb}") for cb in range(CB)]
            stats_sb = [pers.tile([128, 2], f32, name=f"st{cb}", tag=f"st{cb}") for cb in range(CB)]
            mexp_sb = pers.tile([G, 2], f32, name="mexp", tag="mexp")
            var_sb = pers.tile([G, 1], f32, name="var", tag="var")
            lnv_sb = pers.tile([G, 1], f32, name="lnv", tag="lnv")
            negmu_sb = pers.tile([G, 1], f32, name="negmu", tag="negmu")
            eps_sb = pers.tile([G, 1], f32, name="eps", tag="eps")
            rs2_sb = pers.tile([G, 2], f32, name="rs2", tag="rs2")
            a_sb = [pers.tile([128, 1], f32, name=f"a{cb}", tag=f"a{cb}") for cb in range(CB)]
            bvec_sb = [pers.tile([128, 1], f32, name=f"b{cb}", tag=f"b{cb}") for cb in range(CB)]
            expc_sb = pers.tile([128, 1], f32, name="expc", tag="expc")
            nc.sync.dma_start(ones8[:], on8_d[:, :])
            nc.sync.dma_start(ones_row[:], onr_d[:, :])
            nc.gpsimd.memset(eps_sb[:], EPS)
            nc.gpsimd.memset(expc_sb[:], -EXPC)

            for _rep in range(repeats):

                # ---- input DMA (x chunked so stats can start early) ----
                NCH = 4
                CHW = N // NCH
                for ch in range(NCH):
                    for cb in range(CB):
                        eng = nc.sync if cb == 0 else nc.gpsimd
                        eng.dma_start(
                            x_sb[cb][:, ch * CHW : (ch + 1) * CHW],
                            x_d[cb * 128 : (cb + 1) * 128, ch * CHW : (ch + 1) * CHW],
                        )
                nc.sync.dma_start(wq_sb[:], wq_d[:, :, :])
                nc.sync.dma_start(wk_sb[:], wk_d[:, :, :])
                nc.sync.dma_start(wv_sb[:], wv_d[:, :, :])
                nc.sync.dma_start(wp_sb[:], wp_d[:, :, :])
                for cb in range(CB):
                    sl = slice(cb * 128, (cb + 1) * 128)
                    nc.sync.dma_start(gam_sb[cb][:], gam_d[sl, :])
                    nc.sync.dma_start(bet_sb[cb][:], bet_d[sl, :])
                    nc.sync.dma_start(bp_sb[cb][:], bp_d[sl, :])
                    nc.sync.dma_start(g_sb[cb][:], g_d[sl, :])
                    nc.sync.dma_start(gt_sb[cb][:], gt_d[:, sl])

                # ---- GroupNorm statistics ----
                # per-partition sum (VectorE) and sum-of-squares (ScalarE —
                # idle during the prologue; the Square output is dumped into
                # the fp8 scratch tile)
                for cb in range(CB):
                    for ch in range(NCH):
                        xa = x_sb[cb][:, ch * CHW : (ch + 1) * CHW]
                        nc.vector.reduce_sum(
                            psum_part[cb][:, ch : ch + 1], xa, axis=mybir.AxisListType.X
                        )
                        nc.scalar.activation(
                            sq8[:, ch * CHW : (ch + 1) * CHW],
                            xa,
                            AF.Square,
                            accum_out=psq_part[cb][:, ch : ch + 1],
                        )
                    nc.vector.reduce_sum(
                        stats_sb[cb][:, 0:1], psum_part[cb][:], axis=mybir.AxisListType.X
                    )
                    nc.vector.reduce_sum(
                        stats_sb[cb][:, 1:2], psq_part[cb][:], axis=mybir.AxisListType.X
                    )

                with tc.tile_pool(name="ps_gn", bufs=1, space="PSUM") as ps_gn:
                    ps_g = ps_gn.tile([G, 2], f32, name="ps_g", tag="ps_g")
                    for cb in range(CB):
                        nc.tensor.matmul(
                            ps_g[:],
                            g_sb[cb][:],
                            stats_sb[cb][:],
                            start=(cb == 0),
                            stop=(cb == CB - 1),
                        )
                    # copy stats, var = E[x^2] - mu^2 (sign folded), -mu
                    nc.vector.tensor_copy(mexp_sb[:], ps_g[:])
                    nc.vector.scalar_tensor_tensor(
                        var_sb[:], mexp_sb[:, 0:1], mexp_sb[:, 0:1],
                        mexp_sb[:, 1:2], op0=Alu.mult, op1=Alu.subtract,
                    )
                    nc.vector.tensor_scalar_mul(negmu_sb[:], mexp_sb[:, 0:1], -1.0)
                    # rstd = exp(-0.5*ln(var+eps)); rs2b = -mu*rstd
                    nc.scalar.activation(
                        lnv_sb[:], var_sb[:], AF.Ln, bias=eps_sb[:], scale=-1.0
                    )
                    nc.scalar.activation(rs2_sb[:, 0:1], lnv_sb[:], AF.Exp, scale=-0.5)
                    nc.scalar.activation(
                        rs2_sb[:, 1:2], rs2_sb[:, 0:1], AF.Identity, scale=negmu_sb[:]
                    )
                    for cb in range(CB):
                        ps_ab = ps_gn.tile([128, 2], f32, name="ps_ab", tag="ps_ab")
                        nc.tensor.matmul(
                            ps_ab[:], gt_sb[cb][:], rs2_sb[:], start=True, stop=True
                        )
                        nc.vector.tensor_mul(a_sb[cb][:], ps_ab[:, 0:1], gam_sb[cb][:])
                        nc.vector.scalar_tensor_tensor(
                            bvec_sb[cb][:], ps_ab[:, 1:2], gam_sb[cb][:],
                            bet_sb[cb][:], op0=Alu.mult, op1=Alu.add,
                        )

                # ---- xn8 = a*x + b in fp8, chunked; cb0 on DVE, cb1 on
                # GPSIMD so both halves finish early ----
                XCH = 8
                XW = N // XCH
                for xc in range(XCH):
                    xs = slice(xc * XW, (xc + 1) * XW)
                    nc.vector.tensor_scalar(
                        xn8[:, 0, xs], x_sb[0][:, xs],
                        a_sb[0][:], bvec_sb[0][:], op0=Alu.mult, op1=Alu.add,
                    )
                    nc.gpsimd.tensor_scalar(
                        xn8[:, 1, xs], x_sb[1][:, xs],
                        a_sb[1][:], bvec_sb[1][:], op0=Alu.mult, op1=Alu.add,
                    )

                # ---- QKV projections (DoubleRow, K=256 in one pass),
                # interleaved k/q/v per 512-column block so attention can
                # start as soon as the first blocks exist ----
                with tc.tile_pool(name="ps_qkv", bufs=3, space="PSUM") as ps_qkv, \
                     tc.tile_pool(name="ps_v", bufs=2, space="PSUM") as ps_vp:
                    for mt in range(N // 512):
                        ms = slice(mt * 512, (mt + 1) * 512)
                        for ob in range(CB):
                            ps = ps_qkv.tile([128, 512], f32, name="ps_k", tag="ps_k")
                            nc.tensor.matmul(
                                ps[:],
                                wk_sb[:, :, ob * 128 : (ob + 1) * 128],
                                xn8[:, :, ms],
                                start=True, stop=True, perf_mode=DR,
                            )
                            nc.vector.tensor_copy(k8[:, ob, ms], ps[:])
                        if mt < NQ // 512:
                            for ob in range(CB):
                                ps = ps_qkv.tile([128, 512], f32, name="ps_k", tag="ps_k")
                                nc.tensor.matmul(
                                    ps[:],
                                    wq_sb[:, :, ob * 128 : (ob + 1) * 128],
                                    xn8[:, :, ms],
                                    start=True, stop=True, perf_mode=DR,
                                )
                                nc.vector.tensor_copy(q8[:, ob, ms], ps[:])
                        # v for the 4 key blocks in this 512 chunk
                        for mbi in range(4):
                            mb = mt * 4 + mbi
                            ps_v = ps_vp.tile([128, C], f32, name="ps_v", tag="ps_v")
                            nc.tensor.matmul(
                                ps_v[:],
                                xn8[:, :, mb * 128 : (mb + 1) * 128],
                                wv_sb[:],
                                start=True, stop=True, perf_mode=DR,
                            )
                            nc.vector.tensor_copy(vT8[:, mb, :], ps_v[:])

                # ---- attention + projection, tiled over 512-query tiles ----
                with tc.tile_pool(name="pT", bufs=3) as pT_pool, \
                     tc.tile_pool(name="att_sb", bufs=2) as att_sb, \
                     tc.tile_pool(name="y_sb", bufs=2) as y_pool, \
                     tc.tile_pool(name="ps_s", bufs=2, space="PSUM") as ps_s_pool, \
                     tc.tile_pool(name="ps_o", bufs=1, space="PSUM") as ps_o_pool, \
                     tc.tile_pool(name="ps_m", bufs=2, space="PSUM") as ps_m_pool:
                    for nt in range(NT):
                        qs = slice(nt * 512, (nt + 1) * 512)
                        ps_out = [
                            ps_o_pool.tile([128, 512], f32, name=f"ps_out{cb}", tag=f"ps_out{cb}")
                            for cb in range(CB)
                        ]
                        ps_den = ps_m_pool.tile([1, 512], f32, name="ps_den", tag="ps_m")
                        for j in range(PAIRS):
                            ps_s = ps_s_pool.tile([128, 2, 512], f32, name="ps_s", tag="ps_s")
                            for t in range(2):
                                mb = 2 * j + t
                                nc.tensor.matmul(
                                    ps_s[:, t, :],
                                    k8[:, :, mb * 128 : (mb + 1) * 128],
                                    q8[:, :, qs],
                                    start=True, stop=True, perf_mode=DR,
                                )
                            pT = pT_pool.tile([128, 2, 512], f8, name="pT", tag="pT")
                            nc.scalar.activation(
                                pT[:], ps_s[:], AF.Exp, bias=expc_sb[:], scale=SCL
                            )
                            for cb in range(CB):
                                nc.tensor.matmul(
                                    ps_out[cb][:],
                                    vT8[:, 2 * j : 2 * j + 2, cb * 128 : (cb + 1) * 128],
                                    pT[:],
                                    start=(j == 0),
                                    stop=(j == PAIRS - 1),
                                    perf_mode=DR,
                                )
                            nc.tensor.matmul(
                                ps_den[:],
                                ones8[:, :, 0:1],
                                pT[:],
                                start=(j == 0),
                                stop=(j == PAIRS - 1),
                                perf_mode=DR,
                            )
                        # 1/den -> broadcast to 128 partitions via ones matmul
                        r_row = att_sb.tile([1, 512], f32, name="r_row", tag="r_row")
                        nc.vector.reciprocal_approx_fast(r_row[:], ps_den[:])
                        r_rowr = att_sb.tile([1, 512], fr, name="r_rowr", tag="r_rowr")
                        nc.vector.tensor_copy(r_rowr[:], r_row[:])
                        ps_bc = ps_m_pool.tile([128, 512], f32, name="ps_bc", tag="ps_m")
                        nc.tensor.matmul(
                            ps_bc[:], ones_row[:], r_rowr[:], start=True, stop=True
                        )
                        r_bc = att_sb.tile([128, 512], f32, name="r_bc", tag="r_bc")
                        nc.vector.tensor_copy(r_bc[:], ps_bc[:])
                        # evacuate attention output with the 1/den scale and
                        # fp8 quantization fused in
                        out_s8 = att_sb.tile([128, CB, 512], f8, name="out_s8", tag="out_s8")
                        for cb in range(CB):
                            nc.vector.tensor_mul(
                                out_s8[:, cb, :], ps_out[cb][:], r_bc[:]
                            )
                        # final projection (DoubleRow) + bias + residual
                        for ob in range(CB):
                            ps_pj = ps_m_pool.tile([128, 512], f32, name="ps_pj", tag="ps_m")
                            nc.tensor.matmul(
                                ps_pj[:],
                                wp_sb[:, :, ob * 128 : (ob + 1) * 128],
                                out_s8[:],
                                start=True, stop=True, perf_mode=DR,
                            )
                            y2 = y_pool.tile([128, 512], f32, name="y2", tag="y2")
                            nc.vector.scalar_tensor_tensor(
                                y2[:], ps_pj[:], bp_sb[ob][:],
                                x_sb[ob][:, qs], op0=Alu.add, op1=Alu.add,
                            )
                            nc.sync.dma_start(out_d[ob * 128 : (ob + 1) * 128, qs], y2[:])
    nc.compile()
    nc.finalize()
    return nc


def _get_graph():
    global _GRAPH
    if _GRAPH is None:
        _GRAPH = _build_graph()
    return _GRAPH


def _host_inputs(x, gamma, beta, w_qkv, w_proj, b_proj):
    import ml_dtypes

    f = np.float32
    f8 = ml_dtypes.float8_e4m3

    def w8(wT):  # [C, C] (c, o) -> [128, CB, C] fp8
        return np.ascontiguousarray(
            wT.reshape(CB, 128, C).transpose(1, 0, 2).astype(f8)
        )

    wq = w8(np.asarray(w_qkv[0:C], dtype=f).T)
    wk = w8(np.asarray(w_qkv[C : 2 * C], dtype=f).T)
    wv = w8(np.asarray(w_qkv[2 * C : 3 * C], dtype=f).T)
    wp = w8(np.asarray(w_proj, dtype=f).T)
    gm = np.zeros((C, G), dtype=f)
    gm[np.arange(C), np.arange(C) // (C // G)] = 1.0 / float((C // G) * N)
    gmT = np.ascontiguousarray((gm != 0).astype(f).T)
    com = {
        "wq8": wq,
        "wk8": wk,
        "wv8": wv,
        "wp8": wp,
        "gamma": np.ascontiguousarray(gamma.reshape(C, 1), dtype=f),
        "beta": np.ascontiguousarray(beta.reshape(C, 1), dtype=f),
        "b_proj": np.ascontiguousarray(b_proj.reshape(C, 1), dtype=f),
        "G": gm,
        "GT": gmT,
        "ones8": np.ones((128, CB * 16), dtype=f8),
        "ones_row": np.ones((1, 128), dtype=f),
    }
    in_maps = []
    for j in range(8):
        b, h = j // 2, j % 2
        x2 = np.asarray(x[b], dtype=f).reshape(C, N)
        xr = np.ascontiguousarray(np.roll(x2, -h * NQ, axis=1))
        in_maps.append({"x": xr, **com})
    return in_maps


def kernel(x, gamma, beta, w_qkv, w_proj, b_proj):
    from concourse.bass_utils import run_bass_kernel_spmd

    x = np.asarray(x)
    B, _, H, W = x.shape
    nc = _get_graph()
    in_maps = _host_inputs(x, gamma, beta, w_qkv, w_proj, b_proj)
    res = run_bass_kernel_spmd(nc, in_maps, core_ids=list(range(8)))
    y = np.empty((B, C, N), dtype=np.float32)
    for j in range(8):
        b, h = j // 2, j % 2
        y[b][:, h * NQ : (h + 1) * NQ] = res.results[j]["out"]
    return y.reshape(B, C, H, W)


# revision 10
# speedup vs baseline: 1.2652x; 1.0127x over previous
"""Trainium2 Bass kernel: GroupNorm + single-head self-attention block.

Reference computation (per batch element b):
    xn  = GroupNorm(x)                      # [C, N]  C=256, N=4096, 8 groups
    q,k,v = w_qkv @ xn (split)              # each [C, N]
    s   = (q^T k) * C^-0.5                  # [N, N]
    p   = softmax(s, axis=-1)
    out = v @ p^T                           # [C, N]
    y   = x + w_proj @ out + b_proj

Sharding: data-parallel over batch B=4 across 8 cores, 2 cores per batch
element.  Each core handles NQ=2048 of the 4096 queries and redundantly
computes GroupNorm/K/V for its batch element.  SPMD trick: the host rolls
x along N per core so the core's query half is always columns [0, NQ).

fp8 design (v2):
  - q/k/v/xn and all conv1x1 weights are quantized to fp8e4m3 on chip;
    every big matmul runs in DoubleRow perf mode (K=256 contracted in a
    single pass, 0.5 cycles/row) with 3D [128, 2, F] access patterns.
  - scores are computed transposed s_T[m, n] (keys on partitions) via
    matmul(lhsT=k8 block, rhs=q8 tile); PSUM holds the raw q.k dot
    product and the mandatory C^-0.5 scale plus a constant exp-shift
    ride the Exp activation for free:  pT = exp(s/16 - EXPC), emitted
    directly in fp8 (values bounded ~e^4 << 240-sat).  The constant
    shift cancels in the softmax ratio.
  - softmax denominator comes from a ones-lhsT DoubleRow matmul on
    TensorE accumulated across all key pairs (no DVE folds at all);
    1/den via the fast custom-DVE reciprocal, broadcast to 128
    partitions with a tiny ones-column matmul.
  - the 1/den scale is applied during the attention-output PSUM
    evacuation (DVE tensor_mul reading two PSUM tiles), which also
    quantizes to fp8 for the DoubleRow output projection.
  - GroupNorm: per-partition sums on DVE, sum-of-squares via ACT Square
    accum (prologue-idle engine), xn=a*x+b emitted as fp8 split between
    DVE and GPSIMD.
"""

import numpy as np

C = 256
N = 4096
NQ = 2048  # queries per core
G = 8  # groupnorm groups
CB = 2  # channel blocks of 128
NT = NQ // 512  # query tiles per core
MB = N // 128  # key blocks
PAIRS = MB // 2  # key pair-blocks (256 keys each)
EPS = 1e-5
SCL = C ** -0.5  # folded into the Exp activation scale
EXPC = 2.0  # constant exp shift (cancels in softmax); keeps pT in fp8 range

_GRAPH = None


def _build_graph(repeats=1):
    import concourse.bass as bass
    import concourse.mybir as mybir
    from concourse import bacc, tile

    dt = mybir.dt
    f32 = dt.float32
    fr = dt.float32r
    f8 = dt.float8e4
    AF = mybir.ActivationFunctionType
    Alu = mybir.AluOpType
    DR = mybir.MatmulPerfMode.DoubleRow

    nc = bacc.Bacc("TRN2", target_bir_lowering=False, debug=False, num_devices=8)

    x_d = nc.declare_dram_parameter("x", [C, N], f32, isOutput=False)
    wq_d = nc.declare_dram_parameter("wq8", [128, CB, C], f8, isOutput=False)
    wk_d = nc.declare_dram_parameter("wk8", [128, CB, C], f8, isOutput=False)
    wv_d = nc.declare_dram_parameter("wv8", [128, CB, C], f8, isOutput=False)
    wp_d = nc.declare_dram_parameter("wp8", [128, CB, C], f8, isOutput=False)
    gam_d = nc.declare_dram_parameter("gamma", [C, 1], f32, isOutput=False)
    bet_d = nc.declare_dram_parameter("beta", [C, 1], f32, isOutput=False)
    bp_d = nc.declare_dram_parameter("b_proj", [C, 1], f32, isOutput=False)
    g_d = nc.declare_dram_parameter("G", [C, G], f32, isOutput=False)
    on8_d = nc.declare_dram_parameter("ones8", [128, CB * 16], f8, isOutput=False)
    onr_d = nc.declare_dram_parameter("ones_row", [1, 128], fr, isOutput=False)
    gt_d = nc.declare_dram_parameter("GT", [G, C], f32, isOutput=False)
    out_d = nc.declare_dram_parameter("out", [C, NQ], f32, isOutput=True)

    with tile.TileContext(nc) as tc:
        with tc.tile_pool(name="pers", bufs=1) as pers:
            # ---- persistent SBUF tiles ----
            NCH = 8
            CHW = N // NCH  # 512
            x_sb = [
                [
                    pers.tile([128, CHW], f32, name=f"x{cb}_{ch}", tag=f"x{cb}_{ch}")
                    for ch in range(NCH)
                ]
                for cb in range(CB)
            ]
            xn8 = pers.tile([128, CB, N], f8, name="xn8", tag="xn8")
            k8 = pers.tile([128, CB, N], f8, name="k8", tag="k8")
            q8 = pers.tile([128, CB, NQ], f8, name="q8", tag="q8")
            vT8 = pers.tile([128, MB, C], f8, name="vT8", tag="vT8")
            sq8 = pers.tile([128, N], f8, name="sq8", tag="sq8")  # Square dump
            wq_sb = pers.tile([128, CB, C], f8, name="wq8s", tag="wq8s")
            wk_sb = pers.tile([128, CB, C], f8, name="wk8s", tag="wk8s")
            wv_sb = pers.tile([128, CB, C], f8, name="wv8s", tag="wv8s")
            wp_sb = pers.tile([128, CB, C], f8, name="wp8s", tag="wp8s")
            gam_sb = [pers.tile([128, 1], f32, name=f"gam{cb}", tag=f"gam{cb}") for cb in range(CB)]
            bet_sb = [pers.tile([128, 1], f32, name=f"bet{cb}", tag=f"bet{cb}") for cb in range(CB)]
            bp_sb = [pers.tile([128, 1], f32, name=f"bp{cb}", tag=f"bp{cb}") for cb in range(CB)]
            g_sb = [pers.tile([128, G], f32, name=f"g{cb}", tag=f"g{cb}") for cb in range(CB)]
            gt_sb = [pers.tile([G, 128], f32, name=f"gt{cb}", tag=f"gt{cb}") for cb in range(CB)]
            # ones for the denominator matmul: slice [:, :, 0:1] has a
            # 16-byte dim1 stride (DoubleRow alignment requirement)
            ones8 = pers.tile([128, CB, 16], f8, name="ones8", tag="ones8")
            ones_row = pers.tile([1, 128], fr, name="ones_row", tag="ones_row")
            psum_part = [pers.tile([128, NCH], f32, name=f"psm{cb}", tag=f"psm{cb}") for cb in range(CB)]
            psq_part = [pers.tile([128, NCH], f32, name=f"psq{cb}", tag=f"psq{cb}") for cb in range(CB)]
            stats_sb = [pers.tile([128, 2], f32, name=f"st{cb}", tag=f"st{cb}") for cb in range(CB)]
            mexp_sb = pers.tile([G, 2], f32, name="mexp", tag="mexp")
            var_sb = pers.tile([G, 1], f32, name="var", tag="var")
            lnv_sb = pers.tile([G, 1], f32, name="lnv", tag="lnv")
            negmu_sb = pers.tile([G, 1], f32, name="negmu", tag="negmu")
            eps_sb = pers.tile([G, 1], f32, name="eps", tag="eps")
            rs2_sb = pers.tile([G, 2], f32, name="rs2", tag="rs2")
            a_sb = [pers.tile([128, 1], f32, name=f"a{cb}", tag=f"a{cb}") for cb in range(CB)]
            bvec_sb = [pers.tile([128, 1], f32, name=f"b{cb}", tag=f"b{cb}") for cb in range(CB)]
            expc_sb = pers.tile([128, 1], f32, name="expc", tag="expc")
            nc.sync.dma_start(ones8[:], on8_d[:, :])
            nc.sync.dma_start(ones_row[:], onr_d[:, :])
            nc.gpsimd.memset(eps_sb[:], EPS)
            nc.gpsimd.memset(expc_sb[:], -EXPC)

            for _rep in range(repeats):

                # ---- input DMA (x chunked so stats can start early) ----
                for ch in range(NCH):
                    for cb in range(CB):
                        eng = nc.sync if cb == 0 else nc.gpsimd
                        eng.dma_start(
                            x_sb[cb][ch][:],
                            x_d[cb * 128 : (cb + 1) * 128, ch * CHW : (ch + 1) * CHW],
                        )
                nc.sync.dma_start(wq_sb[:], wq_d[:, :, :])
                nc.sync.dma_start(wk_sb[:], wk_d[:, :, :])
                nc.sync.dma_start(wv_sb[:], wv_d[:, :, :])
                nc.sync.dma_start(wp_sb[:], wp_d[:, :, :])
                for cb in range(CB):
                    sl = slice(cb * 128, (cb + 1) * 128)
                    nc.sync.dma_start(gam_sb[cb][:], gam_d[sl, :])
                    nc.sync.dma_start(bet_sb[cb][:], bet_d[sl, :])
                    nc.sync.dma_start(bp_sb[cb][:], bp_d[sl, :])
                    nc.sync.dma_start(g_sb[cb][:], g_d[sl, :])
                    nc.sync.dma_start(gt_sb[cb][:], gt_d[:, sl])

                # ---- PE warm-up: the HAM clock gate keeps the PE at 1.2GHz
                # until it sees ~3.4us of sustained activity.  Burn the DMA
                # wait on dummy matmuls so the real stream runs at 2.4GHz
                # from its first instruction. ----
                with tc.tile_pool(name="ps_w", bufs=1, space="PSUM") as ps_wp:
                    ps_warm = ps_wp.tile([1, C], f32, name="ps_warm", tag="ps_warm")
                    for _w in range(72):
                        nc.tensor.matmul(
                            ps_warm[:],
                            ones8[:, :, 0:1],
                            wq_sb[:],
                            start=True, stop=True, perf_mode=DR,
                        )
                    warm_junk = pers.tile([1, C], f32, name="warm_junk", tag="warm_junk")
                    nc.vector.tensor_copy(warm_junk[:], ps_warm[:])

                # ---- GroupNorm statistics ----
                # per-partition sum (VectorE) and sum-of-squares (ScalarE --
                # idle during the prologue; the Square output is dumped into
                # the fp8 scratch tile)
                for cb in range(CB):
                    for ch in range(NCH):
                        xa = x_sb[cb][ch][:]
                        nc.vector.reduce_sum(
                            psum_part[cb][:, ch : ch + 1], xa, axis=mybir.AxisListType.X
                        )
                        nc.scalar.activation(
                            sq8[:, ch * CHW : (ch + 1) * CHW],
                            xa,
                            AF.Square,
                            accum_out=psq_part[cb][:, ch : ch + 1],
                        )
                    nc.vector.reduce_sum(
                        stats_sb[cb][:, 0:1], psum_part[cb][:], axis=mybir.AxisListType.X
                    )
                    nc.vector.reduce_sum(
                        stats_sb[cb][:, 1:2], psq_part[cb][:], axis=mybir.AxisListType.X
                    )

                with tc.tile_pool(name="ps_gn", bufs=1, space="PSUM") as ps_gn:
                    ps_g = ps_gn.tile([G, 2], f32, name="ps_g", tag="ps_g")
                    for cb in range(CB):
                        nc.tensor.matmul(
                            ps_g[:],
                            g_sb[cb][:],
                            stats_sb[cb][:],
                            start=(cb == 0),
                            stop=(cb == CB - 1),
                        )
                    # copy stats, var = E[x^2] - mu^2 (sign folded), -mu
                    nc.vector.tensor_copy(mexp_sb[:], ps_g[:])
                    nc.vector.scalar_tensor_tensor(
                        var_sb[:], mexp_sb[:, 0:1], mexp_sb[:, 0:1],
                        mexp_sb[:, 1:2], op0=Alu.mult, op1=Alu.subtract,
                    )
                    nc.vector.tensor_scalar_mul(negmu_sb[:], mexp_sb[:, 0:1], -1.0)
                    # rstd = exp(-0.5*ln(var+eps)); rs2b = -mu*rstd
                    nc.scalar.activation(
                        lnv_sb[:], var_sb[:], AF.Ln, bias=eps_sb[:], scale=-1.0
                    )
                    nc.scalar.activation(rs2_sb[:, 0:1], lnv_sb[:], AF.Exp, scale=-0.5)
                    nc.scalar.activation(
                        rs2_sb[:, 1:2], rs2_sb[:, 0:1], AF.Identity, scale=negmu_sb[:]
                    )
                    for cb in range(CB):
                        ps_ab = ps_gn.tile([128, 2], f32, name="ps_ab", tag="ps_ab")
                        nc.tensor.matmul(
                            ps_ab[:], gt_sb[cb][:], rs2_sb[:], start=True, stop=True
                        )
                        nc.vector.tensor_mul(a_sb[cb][:], ps_ab[:, 0:1], gam_sb[cb][:])
                        nc.vector.scalar_tensor_tensor(
                            bvec_sb[cb][:], ps_ab[:, 1:2], gam_sb[cb][:],
                            bet_sb[cb][:], op0=Alu.mult, op1=Alu.add,
                        )

                # ---- xn8 = a*x + b in fp8, chunked; cb0 on DVE, cb1 on
                # GPSIMD so both halves finish early ----
                for xc in range(NCH):
                    xs = slice(xc * CHW, (xc + 1) * CHW)
                    nc.vector.tensor_scalar(
                        xn8[:, 0, xs], x_sb[0][xc][:],
                        a_sb[0][:], bvec_sb[0][:], op0=Alu.mult, op1=Alu.add,
                    )
                    nc.gpsimd.tensor_scalar(
                        xn8[:, 1, xs], x_sb[1][xc][:],
                        a_sb[1][:], bvec_sb[1][:], op0=Alu.mult, op1=Alu.add,
                    )

                # ---- QKV projections (DoubleRow, K=256 in one pass),
                # interleaved k/q/v per 512-column block so attention can
                # start as soon as the first blocks exist ----
                with tc.tile_pool(name="ps_qkv", bufs=3, space="PSUM") as ps_qkv, \
                     tc.tile_pool(name="ps_v", bufs=2, space="PSUM") as ps_vp:
                    for mt in range(N // 512):
                        ms = slice(mt * 512, (mt + 1) * 512)
                        for ob in range(CB):
                            ps = ps_qkv.tile([128, 512], f32, name="ps_k", tag="ps_k")
                            nc.tensor.matmul(
                                ps[:],
                                wk_sb[:, :, ob * 128 : (ob + 1) * 128],
                                xn8[:, :, ms],
                                start=True, stop=True, perf_mode=DR,
                            )
                            nc.vector.tensor_copy(k8[:, ob, ms], ps[:])
                        if mt < NQ // 512:
                            for ob in range(CB):
                                ps = ps_qkv.tile([128, 512], f32, name="ps_k", tag="ps_k")
                                nc.tensor.matmul(
                                    ps[:],
                                    wq_sb[:, :, ob * 128 : (ob + 1) * 128],
                                    xn8[:, :, ms],
                                    start=True, stop=True, perf_mode=DR,
                                )
                                nc.vector.tensor_copy(q8[:, ob, ms], ps[:])
                        # v for the 4 key blocks in this 512 chunk
                        for mbi in range(4):
                            mb = mt * 4 + mbi
                            ps_v = ps_vp.tile([128, C], f32, name="ps_v", tag="ps_v")
                            nc.tensor.matmul(
                                ps_v[:],
                                xn8[:, :, mb * 128 : (mb + 1) * 128],
                                wv_sb[:],
                                start=True, stop=True, perf_mode=DR,
                            )
                            nc.vector.tensor_copy(vT8[:, mb, :], ps_v[:])

                # ---- attention + projection: software-pipelined over all
                # (nt, pair) steps.  scores/exp run LAG pairs ahead of
                # attnout/den; each tile's softmax/projection tail is
                # emitted inside the next tile's early score phase so no
                # engine drains at tile boundaries. ----
                LAG = 2
                with tc.tile_pool(name="pT", bufs=4) as pT_pool, \
                     tc.tile_pool(name="att_sb", bufs=2) as att_sb, \
                     tc.tile_pool(name="y_sb", bufs=2) as y_pool, \
                     tc.tile_pool(name="ps_s", bufs=2, space="PSUM") as ps_s_pool, \
                     tc.tile_pool(name="ps_o", bufs=1, space="PSUM") as ps_o_pool, \
                     tc.tile_pool(name="ps_m", bufs=2, space="PSUM") as ps_m_pool:
                    seq = [(nt, j) for nt in range(NT) for j in range(PAIRS)]
                    ps_out = {}
                    ps_den = {}
                    pT_t = {}

                    def tail(nt):
                        qs = slice(nt * 512, (nt + 1) * 512)
                        r_row = att_sb.tile([1, 512], f32, name="r_row", tag="r_row")
                        nc.vector.reciprocal_approx_fast(r_row[:], ps_den[nt][:])
                        r_rowr = att_sb.tile([1, 512], fr, name="r_rowr", tag="r_rowr")
                        nc.vector.tensor_copy(r_rowr[:], r_row[:])
                        ps_bc = ps_m_pool.tile([128, 512], f32, name="ps_bc", tag="ps_m")
                        nc.tensor.matmul(
                            ps_bc[:], ones_row[:], r_rowr[:], start=True, stop=True
                        )
                        r_bc = att_sb.tile([128, 512], f32, name="r_bc", tag="r_bc")
                        nc.vector.tensor_copy(r_bc[:], ps_bc[:])
                        # evacuate attention output with the 1/den scale and
                        # fp8 quantization fused in
                        out_s8 = att_sb.tile([128, CB, 512], f8, name="out_s8", tag="out_s8")
                        for cb in range(CB):
                            nc.vector.tensor_mul(
                                out_s8[:, cb, :], ps_out[nt][cb][:], r_bc[:]
                            )
                        # final projection (DoubleRow) + bias + residual
                        for ob in range(CB):
                            ps_pj = ps_m_pool.tile([128, 512], f32, name="ps_pj", tag="ps_m")
                            nc.tensor.matmul(
                                ps_pj[:],
                                wp_sb[:, :, ob * 128 : (ob + 1) * 128],
                                out_s8[:],
                                start=True, stop=True, perf_mode=DR,
                            )
                            y2 = y_pool.tile([128, 512], f32, name="y2", tag="y2")
                            nc.vector.scalar_tensor_tensor(
                                y2[:], ps_pj[:], bp_sb[ob][:],
                                x_sb[ob][nt][:], op0=Alu.add, op1=Alu.add,
                            )
                            nc.sync.dma_start(out_d[ob * 128 : (ob + 1) * 128, qs], y2[:])

                    def attnout_den(nt2, j2):
                        if j2 == 0:
                            ps_out[nt2] = [
                                ps_o_pool.tile(
                                    [128, 512], f32, name=f"ps_out{cb}", tag=f"ps_out{cb}"
                                )
                                for cb in range(CB)
                            ]
                            ps_den[nt2] = ps_m_pool.tile(
                                [1, 512], f32, name="ps_den", tag="ps_m"
                            )
                        pT = pT_t.pop((nt2, j2))
                        for cb in range(CB):
                            nc.tensor.matmul(
                                ps_out[nt2][cb][:],
                                vT8[:, 2 * j2 : 2 * j2 + 2, cb * 128 : (cb + 1) * 128],
                                pT[:],
                                start=(j2 == 0),
                                stop=(j2 == PAIRS - 1),
                                perf_mode=DR,
                            )
                        nc.tensor.matmul(
                            ps_den[nt2][:],
                            ones8[:, :, 0:1],
                            pT[:],
                            start=(j2 == 0),
                            stop=(j2 == PAIRS - 1),
                            perf_mode=DR,
                        )

                    for idx, (nt, j) in enumerate(seq):
                        qs = slice(nt * 512, (nt + 1) * 512)
                        ps_s = ps_s_pool.tile([128, 2, 512], f32, name="ps_s", tag="ps_s")
                        for t in range(2):
                            mb = 2 * j + t
                            nc.tensor.matmul(
                                ps_s[:, t, :],
                                k8[:, :, mb * 128 : (mb + 1) * 128],
                                q8[:, :, qs],
                                start=True, stop=True, perf_mode=DR,
                            )
                        pT = pT_pool.tile([128, 2, 512], f8, name="pT", tag="pT")
                        nc.scalar.activation(
                            pT[:], ps_s[:], AF.Exp, bias=expc_sb[:], scale=SCL
                        )
                        pT_t[(nt, j)] = pT
                        if j == 2 and nt > 0:
                            tail(nt - 1)
                        if idx >= LAG:
                            attnout_den(*seq[idx - LAG])
                    for nt2, j2 in seq[-LAG:]:
                        attnout_den(nt2, j2)
                    tail(NT - 1)
    nc.compile()
    nc.finalize()
    return nc


def _get_graph():
    global _GRAPH
    if _GRAPH is None:
        _GRAPH = _build_graph()
    return _GRAPH


def _host_inputs(x, gamma, beta, w_qkv, w_proj, b_proj):
    import ml_dtypes

    f = np.float32
    f8 = ml_dtypes.float8_e4m3

    def w8(wT):  # [C, C] (c, o) -> [128, CB, C] fp8
        return np.ascontiguousarray(
            wT.reshape(CB, 128, C).transpose(1, 0, 2).astype(f8)
        )

    wq = w8(np.asarray(w_qkv[0:C], dtype=f).T)
    wk = w8(np.asarray(w_qkv[C : 2 * C], dtype=f).T)
    wv = w8(np.asarray(w_qkv[2 * C : 3 * C], dtype=f).T)
    wp = w8(np.asarray(w_proj, dtype=f).T)
    gm = np.zeros((C, G), dtype=f)
    gm[np.arange(C), np.arange(C) // (C // G)] = 1.0 / float((C // G) * N)
    gmT = np.ascontiguousarray((gm != 0).astype(f).T)
    com = {
        "wq8": wq,
        "wk8": wk,
        "wv8": wv,
        "wp8": wp,
        "gamma": np.ascontiguousarray(gamma.reshape(C, 1), dtype=f),
        "beta": np.ascontiguousarray(beta.reshape(C, 1), dtype=f),
        "b_proj": np.ascontiguousarray(b_proj.reshape(C, 1), dtype=f),
        "G": gm,
        "GT": gmT,
        "ones8": np.ones((128, CB * 16), dtype=f8),
        "ones_row": np.ones((1, 128), dtype=f),
    }
    in_maps = []
    for j in range(8):
        b, h = j // 2, j % 2
        x2 = np.asarray(x[b], dtype=f).reshape(C, N)
        xr = np.ascontiguousarray(np.roll(x2, -h * NQ, axis=1))
        in_maps.append({"x": xr, **com})
    return in_maps


def kernel(x, gamma, beta, w_qkv, w_proj, b_proj):
    from concourse.bass_utils import run_bass_kernel_spmd

    x = np.asarray(x)
    B, _, H, W = x.shape
    nc = _get_graph()
    in_maps = _host_inputs(x, gamma, beta, w_qkv, w_proj, b_proj)
    res = run_bass_kernel_spmd(nc, in_maps, core_ids=list(range(8)))
    y = np.empty((B, C, N), dtype=np.float32)
    for j in range(8):
        b, h = j // 2, j % 2
        y[b][:, h * NQ : (h + 1) * NQ] = res.results[j]["out"]
    return y.reshape(B, C, H, W)
